# revision 5
# baseline (speedup 1.0000x reference)
"""Trainium2 Bass kernel for nn_LocalFeatureEncoder — v2 (transposed dataflow).

Computes, for B=8 batches on 8 NeuronCores (batch b -> core b):
    g      = concat(shape_code, structure_code, pose_code)      # (B, 128)
    local  = einsum('kfz,bz->bkf', W, g) + bias                 # (B, 24, 64)
    out    = einsum('btk,bkf->btf', lbs_weights, local)         # (B, 32768, 64)

Host pre-transposes lbs to lbsT [56, 16384] bf16 (T-half 0 of lbs^T on
rows 0..24, zero pad, T-half 1 on rows 32..56); output is produced as out^T (64, 32768) f32
(f on partitions) and transposed back on host. Device program:

  Stage 1 (overlapped with the lbs stream, no DMA after the const loads):
  24 PE matvecs with per-joint W tiles laid out so PSUM collects
  localcol [64, 24] = local^T in (f-partition, k-col) layout, DVE bias-add,
  one PE transpose -> localT [24, 64], cast-copy to bf16 SBUF.

  Stage 2: per 512-col chunk, TWO matmuls (one per T-half, operand
  partition bases 0/32) fill psum[128, 512] (partitions 0..63 = f half 0,
  64..127 = f half 1), PSUM->SBUF copies alternating DVE/Act, 16 output
  DMAs split over the SP/Act queues with ramped staging-group sizes. A
  single PE warmup matmul keeps the pstate ramp warm while the localT
  copy completes.
"""

import os
from contextlib import ExitStack

import numpy as np
import ml_dtypes

import concourse.bass as bass
import concourse.bacc as bacc
import concourse.tile as tile
from concourse import mybir
from concourse import bass_utils

BF16 = ml_dtypes.bfloat16

B, T, K, Z, F = 8, 32768, 24, 128, 64
TH = T // 2             # 16384 t-cols per half
KK = 2 * K              # 48 partitions of lbsT
CW = 512                # t-cols per matmul (1 PSUM bank)
NC = TH // CW           # 32 chunks
C_BIAS, C_ID = 0, 24    # f32 const layout: [biasT 24 | ident 64]
C_TOT = 88
_GROUPS = (2, 3, 3, 4, 5, 5, 5, 5)   # staging group sizes, sum = NC
_BOUNDS = (0, 1024, 6144, 11264, TH)  # lbs input chunk boundaries (cols)
_SPLIT_IN = False  # True: 2 DMAs/chunk skipping pad rows (less bytes, more issues)

_built = {}


def _build(key=0):
    if key in _built:
        return _built[key]

    f32 = mybir.dt.float32
    bf16 = mybir.dt.bfloat16
    nc = bacc.Bacc("TRN2", target_bir_lowering=False, debug=False)

    lbst_d = nc.dram_tensor("lbst", (56, TH), bf16, kind="ExternalInput")
    wtg_d = nc.dram_tensor("wtg", (128, K * F + 1), bf16, kind="ExternalInput")
    cst_d = nc.dram_tensor("cst", (64, C_TOT), f32, kind="ExternalInput")
    out_d = nc.dram_tensor("out", (F, T), f32, kind="ExternalOutput")

    with tile.TileContext(nc) as tc, ExitStack() as ctx:
        const = ctx.enter_context(tc.tile_pool(name="const", bufs=1))
        big = ctx.enter_context(tc.tile_pool(name="big", bufs=1))
        ps1 = ctx.enter_context(
            tc.tile_pool(name="ps1", bufs=1, space=bass.MemorySpace.PSUM)
        )
        psO = ctx.enter_context(
            tc.tile_pool(name="psO", bufs=6, space=bass.MemorySpace.PSUM)
        )
        stag_pool = ctx.enter_context(tc.tile_pool(name="stag_pool", bufs=6))

        # ---- input DMAs (SP queue): consts first, then lbs chunks ----
        wtg = const.tile([128, K * F + 1], bf16)
        nc.sync.dma_start(wtg[:], wtg_d.ap())
        cst = const.tile([64, C_TOT], f32)
        nc.scalar.dma_start(cst[:], cst_d.ap())

        # first chunk small so stage 2 can start early; host pads lbsT to 56
        # rows (T-half 0 on partitions 0..24, zeros 24..32, half 1 on 32..56 —
        # PE operand bases must be 0/32/64) so each chunk is a single DMA
        lbst_sb = big.tile([56, TH], bf16)
        bounds = list(_BOUNDS)
        for c in range(len(bounds) - 1):
            lo, hi = bounds[c], bounds[c + 1]
            if _SPLIT_IN:
                nc.sync.dma_start(lbst_sb[0:K, lo:hi], lbst_d.ap()[0:K, lo:hi])
                nc.sync.dma_start(
                    lbst_sb[32:32 + K, lo:hi], lbst_d.ap()[32:32 + K, lo:hi]
                )
            else:
                nc.sync.dma_start(lbst_sb[:, lo:hi], lbst_d.ap()[:, lo:hi])

        wt = wtg[:, 0:K * F]              # tile k: [128, F] = W[k].T
        g_col = wtg[:, K * F:K * F + 1]
        biasT = cst[:, C_BIAS:C_BIAS + K]  # [64, 24] = bias^T
        ident = cst[:, C_ID:C_ID + 64]     # [64, 64]

        # ---- stage 1: localT = (W @ g + bias)^T -> [24, 64] bf16, no DMA ----
        lc_ps = ps1.tile([64, K], f32, tag="s1")
        for j in range(K):
            nc.tensor.matmul(
                lc_ps[:, j:j + 1], wt[:, j * F:(j + 1) * F], g_col,
                start=True, stop=True,
            )
        # bias-add writes the local^T columns twice (cols 0..24 and 32..56) so
        # a single base-0 transpose yields localT duplicated at partition
        # bases 0 and 32 — each T-half matmul then has base-aligned lhsT/rhs
        localcol = const.tile([64, 32 + K], f32)
        nc.vector.memset(localcol[:, K:32], 0.0)
        nc.vector.tensor_add(localcol[:, 0:K], lc_ps[:], biasT)
        nc.vector.tensor_add(localcol[:, 32:32 + K], lc_ps[:], biasT)

        lT_ps = ps1.tile([32 + K, F], f32, tag="s1")
        nc.tensor.transpose(lT_ps[:], localcol[:], ident)

        # keep the PE busy while the localT copy completes so the pstate ramp
        # (low->mid->max after 3us continuous) is warm when stage 2 starts
        warm_ps = ps1.tile([128, CW], f32, tag="warm")
        nc.tensor.matmul(
            warm_ps[:], wtg[:, 0:128], wtg[:, 0:CW], start=True, stop=True,
        )

        localT = const.tile([32 + K, F], bf16)
        nc.vector.tensor_copy(localT[:], lT_ps[:])

        # ---- stage 2: 64 matmuls + 32 copies + 16 output DMAs ----
        # ramped group sizes: small groups first to minimize pipeline-fill
        # latency, large groups in steady state to minimize DMA issue count
        group_sizes = list(_GROUPS)
        assert sum(group_sizes) == NC
        c = 0
        for gi, gs in enumerate(group_sizes):
            stag = stag_pool.tile([128, gs * CW], f32)
            lo = c
            for i in range(gs):
                ops = psO.tile([128, CW], f32)
                nc.tensor.matmul(
                    ops[0:F, :], localT[0:K, :],
                    lbst_sb[0:K, c * CW:(c + 1) * CW],
                    start=True, stop=True,
                )
                nc.tensor.matmul(
                    ops[F:2 * F, :], localT[32:32 + K, :],
                    lbst_sb[32:32 + K, c * CW:(c + 1) * CW],
                    start=True, stop=True,
                )
                if c % 2 == 0:
                    nc.vector.tensor_copy(stag[:, i * CW:(i + 1) * CW], ops[:])
                else:
                    nc.scalar.copy(stag[:, i * CW:(i + 1) * CW], ops[:])
                c += 1
            h0 = out_d.ap()[:, lo * CW:c * CW]
            h1 = out_d.ap()[:, TH + lo * CW:TH + c * CW]
            if gi % 2 == 0:
                nc.sync.dma_start(h0, stag[0:F, :])
                nc.scalar.dma_start(h1, stag[F:2 * F, :])
            else:
                nc.scalar.dma_start(h0, stag[0:F, :])
                nc.sync.dma_start(h1, stag[F:2 * F, :])

    nc.compile()
    _built[key] = nc
    return nc


def make_in_maps(inputs):
    g_full = np.concatenate(
        [inputs["shape_code"], inputs["structure_code"], inputs["pose_code"]],
        axis=-1,
    ).astype(np.float32)  # (8, 128)
    W = inputs["W"].astype(np.float32)
    # wt[z, k*F + f] = W[k, f, z]
    wt = np.ascontiguousarray(W.transpose(2, 0, 1).reshape(Z, K * F))
    cst = np.zeros((64, C_TOT), dtype=np.float32)
    cst[:, C_BIAS:C_BIAS + K] = inputs["bias"].astype(np.float32).T
    cst[:, C_ID:C_ID + 64] = np.eye(64, dtype=np.float32)
    lbs_bf = inputs["lbs_weights"].astype(BF16)
    in_maps = []
    for b in range(B):
        wtg = np.empty((128, K * F + 1), dtype=BF16)
        wtg[:, 0:K * F] = wt.astype(BF16)
        wtg[:, K * F] = g_full[b].astype(BF16)
        lbst2 = lbs_bf[b].reshape(2, TH, K).transpose(0, 2, 1)  # (2, K, TH)
        lbst = np.zeros((56, TH), dtype=BF16)
        lbst[0:K] = lbst2[0]
        lbst[32:32 + K] = lbst2[1]
        in_maps.append({"lbst": lbst, "wtg": wtg, "cst": cst})
    return in_maps


LAST_RESULT = None


def kernel(**inputs) -> np.ndarray:
    global LAST_RESULT
    inputs = {k: np.asarray(v) for k, v in inputs.items()}
    nc = _build()
    in_maps = make_in_maps(inputs)
    res = bass_utils.run_bass_kernel_spmd(
        nc,
        in_maps,
        core_ids=list(range(B)),
        trace=os.environ.get("LFE_TRACE", "0") == "1",
    )
    LAST_RESULT = res
    out = np.ascontiguousarray(
        np.stack([res.results[b]["out"] for b in range(B)], axis=0)
        .transpose(0, 2, 1)
    )
    return out


if __name__ == "__main__":
    rng = np.random.default_rng(0)
    inputs = {
        "shape_code": rng.standard_normal((B, 64), dtype=np.float32),
        "structure_code": rng.standard_normal((B, 32), dtype=np.float32),
        "pose_code": rng.standard_normal((B, 32), dtype=np.float32),
        "lbs_weights": rng.random((B, T, K), dtype=np.float32),
        "W": rng.standard_normal((K, F, Z), dtype=np.float32),
        "bias": rng.standard_normal((K, F), dtype=np.float32),
    }
    out = kernel(**inputs)
    g = np.concatenate(
        [inputs["shape_code"], inputs["structure_code"], inputs["pose_code"]], -1
    )
    local = np.einsum("kfz,bz->bkf", inputs["W"], g) + inputs["bias"][None]
    ref = np.einsum("btk,bkf->btf", inputs["lbs_weights"], local)
    err = np.abs(out - ref).max() / np.abs(ref).max()
    print("rel err:", err)


# revision 6
# speedup vs baseline: 1.0028x; 1.0028x over previous
"""Trainium2 Bass kernel for nn_LocalFeatureEncoder — v2 (transposed dataflow).

Computes, for B=8 batches on 8 NeuronCores (batch b -> core b):
    g      = concat(shape_code, structure_code, pose_code)      # (B, 128)
    local  = einsum('kfz,bz->bkf', W, g) + bias                 # (B, 24, 64)
    out    = einsum('btk,bkf->btf', lbs_weights, local)         # (B, 32768, 64)

Host pre-transposes lbs to lbsT [56, 16384] bf16 (T-half 0 of lbs^T on
rows 0..24, zero pad, T-half 1 on rows 32..56); output is produced as out^T (64, 32768) f32
(f on partitions) and transposed back on host. Device program:

  Stage 1 (overlapped with the lbs stream, no DMA after the const loads):
  24 PE matvecs with per-joint W tiles laid out so PSUM collects
  localcol [64, 24] = local^T in (f-partition, k-col) layout, DVE bias-add,
  one PE transpose -> localT [24, 64], cast-copy to bf16 SBUF.

  Stage 2: per 512-col chunk, TWO matmuls (one per T-half, operand
  partition bases 0/32) fill psum[128, 512] (partitions 0..63 = f half 0,
  64..127 = f half 1), PSUM->SBUF copies alternating DVE/Act, 16 output
  DMAs split over the SP/Act queues with ramped staging-group sizes. A
  single PE warmup matmul keeps the pstate ramp warm while the localT
  copy completes.
"""

import os
from contextlib import ExitStack

import numpy as np
import ml_dtypes

import concourse.bass as bass
import concourse.bacc as bacc
import concourse.tile as tile
from concourse import mybir
from concourse import bass_utils

BF16 = ml_dtypes.bfloat16

B, T, K, Z, F = 8, 32768, 24, 128, 64
TH = T // 2             # 16384 t-cols per half
KK = 2 * K              # 48 partitions of lbsT
CW = 512                # t-cols per matmul (1 PSUM bank)
NC = TH // CW           # 32 chunks
C_BIAS, C_ID = 0, 24    # f32 const layout: [biasT 24 | ident 64]
C_TOT = 88
_GROUPS = (2, 3, 3, 4, 4, 4, 4, 4, 4)  # staging group sizes, sum = NC
_BOUNDS = (0, 1024, 6144, 11264, TH)  # lbs input chunk boundaries (cols)
_SPLIT_IN = False  # True: 2 DMAs/chunk skipping pad rows (less bytes, more issues)

_built = {}


def _build(key=0):
    if key in _built:
        return _built[key]

    f32 = mybir.dt.float32
    bf16 = mybir.dt.bfloat16
    nc = bacc.Bacc("TRN2", target_bir_lowering=False, debug=False)

    lbst_d = nc.dram_tensor("lbst", (56, TH), bf16, kind="ExternalInput")
    wtg_d = nc.dram_tensor("wtg", (128, K * F + 1), bf16, kind="ExternalInput")
    cst_d = nc.dram_tensor("cst", (64, C_TOT), f32, kind="ExternalInput")
    out_d = nc.dram_tensor("out", (F, T), f32, kind="ExternalOutput")

    with tile.TileContext(nc) as tc, ExitStack() as ctx:
        const = ctx.enter_context(tc.tile_pool(name="const", bufs=1))
        big = ctx.enter_context(tc.tile_pool(name="big", bufs=1))
        ps1 = ctx.enter_context(
            tc.tile_pool(name="ps1", bufs=1, space=bass.MemorySpace.PSUM)
        )
        psO = ctx.enter_context(
            tc.tile_pool(name="psO", bufs=6, space=bass.MemorySpace.PSUM)
        )
        stag_pool = ctx.enter_context(tc.tile_pool(name="stag_pool", bufs=6))

        # ---- input DMAs (SP queue): consts first, then lbs chunks ----
        wtg = const.tile([128, K * F + 1], bf16)
        nc.sync.dma_start(wtg[:], wtg_d.ap())
        cst = const.tile([64, C_TOT], f32)
        nc.scalar.dma_start(cst[:], cst_d.ap())

        # first chunk small so stage 2 can start early; host pads lbsT to 56
        # rows (T-half 0 on partitions 0..24, zeros 24..32, half 1 on 32..56 —
        # PE operand bases must be 0/32/64) so each chunk is a single DMA
        lbst_sb = big.tile([56, TH], bf16)
        bounds = list(_BOUNDS)
        for c in range(len(bounds) - 1):
            lo, hi = bounds[c], bounds[c + 1]
            if _SPLIT_IN:
                nc.sync.dma_start(lbst_sb[0:K, lo:hi], lbst_d.ap()[0:K, lo:hi])
                nc.sync.dma_start(
                    lbst_sb[32:32 + K, lo:hi], lbst_d.ap()[32:32 + K, lo:hi]
                )
            else:
                nc.sync.dma_start(lbst_sb[:, lo:hi], lbst_d.ap()[:, lo:hi])

        wt = wtg[:, 0:K * F]              # tile k: [128, F] = W[k].T
        g_col = wtg[:, K * F:K * F + 1]
        biasT = cst[:, C_BIAS:C_BIAS + K]  # [64, 24] = bias^T
        ident = cst[:, C_ID:C_ID + 64]     # [64, 64]

        # ---- stage 1: localT = (W @ g + bias)^T -> [24, 64] bf16, no DMA ----
        lc_ps = ps1.tile([64, K], f32, tag="s1")
        for j in range(K):
            nc.tensor.matmul(
                lc_ps[:, j:j + 1], wt[:, j * F:(j + 1) * F], g_col,
                start=True, stop=True,
            )
        # bias-add writes the local^T columns twice (cols 0..24 and 32..56) so
        # a single base-0 transpose yields localT duplicated at partition
        # bases 0 and 32 — each T-half matmul then has base-aligned lhsT/rhs
        localcol = const.tile([64, 32 + K], f32)
        nc.vector.memset(localcol[:, K:32], 0.0)
        nc.vector.tensor_add(localcol[:, 0:K], lc_ps[:], biasT)
        nc.vector.tensor_add(localcol[:, 32:32 + K], lc_ps[:], biasT)

        lT_ps = ps1.tile([32 + K, F], f32, tag="s1")
        nc.tensor.transpose(lT_ps[:], localcol[:], ident)

        # keep the PE busy while the localT copy completes so the pstate ramp
        # (low->mid->max after 3us continuous) is warm when stage 2 starts
        warm_ps = ps1.tile([128, CW], f32, tag="warm")
        nc.tensor.matmul(
            warm_ps[:], wtg[:, 0:128], wtg[:, 0:CW], start=True, stop=True,
        )

        localT = const.tile([32 + K, F], bf16)
        nc.vector.tensor_copy(localT[:], lT_ps[:])

        # ---- stage 2: 64 matmuls + 32 copies + 16 output DMAs ----
        # ramped group sizes: small groups first to minimize pipeline-fill
        # latency, large groups in steady state to minimize DMA issue count
        group_sizes = list(_GROUPS)
        assert sum(group_sizes) == NC
        c = 0
        for gi, gs in enumerate(group_sizes):
            stag = stag_pool.tile([128, gs * CW], f32)
            lo = c
            for i in range(gs):
                ops = psO.tile([128, CW], f32)
                nc.tensor.matmul(
                    ops[0:F, :], localT[0:K, :],
                    lbst_sb[0:K, c * CW:(c + 1) * CW],
                    start=True, stop=True,
                )
                nc.tensor.matmul(
                    ops[F:2 * F, :], localT[32:32 + K, :],
                    lbst_sb[32:32 + K, c * CW:(c + 1) * CW],
                    start=True, stop=True,
                )
                if c % 2 == 0:
                    nc.vector.tensor_copy(stag[:, i * CW:(i + 1) * CW], ops[:])
                else:
                    nc.scalar.copy(stag[:, i * CW:(i + 1) * CW], ops[:])
                c += 1
            h0 = out_d.ap()[:, lo * CW:c * CW]
            h1 = out_d.ap()[:, TH + lo * CW:TH + c * CW]
            if gi % 2 == 0:
                nc.sync.dma_start(h0, stag[0:F, :])
                nc.scalar.dma_start(h1, stag[F:2 * F, :])
            else:
                nc.scalar.dma_start(h0, stag[0:F, :])
                nc.sync.dma_start(h1, stag[F:2 * F, :])

    nc.compile()
    _built[key] = nc
    return nc


def make_in_maps(inputs):
    g_full = np.concatenate(
        [inputs["shape_code"], inputs["structure_code"], inputs["pose_code"]],
        axis=-1,
    ).astype(np.float32)  # (8, 128)
    W = inputs["W"].astype(np.float32)
    # wt[z, k*F + f] = W[k, f, z]
    wt = np.ascontiguousarray(W.transpose(2, 0, 1).reshape(Z, K * F))
    cst = np.zeros((64, C_TOT), dtype=np.float32)
    cst[:, C_BIAS:C_BIAS + K] = inputs["bias"].astype(np.float32).T
    cst[:, C_ID:C_ID + 64] = np.eye(64, dtype=np.float32)
    lbs_bf = inputs["lbs_weights"].astype(BF16)
    in_maps = []
    for b in range(B):
        wtg = np.empty((128, K * F + 1), dtype=BF16)
        wtg[:, 0:K * F] = wt.astype(BF16)
        wtg[:, K * F] = g_full[b].astype(BF16)
        lbst2 = lbs_bf[b].reshape(2, TH, K).transpose(0, 2, 1)  # (2, K, TH)
        lbst = np.zeros((56, TH), dtype=BF16)
        lbst[0:K] = lbst2[0]
        lbst[32:32 + K] = lbst2[1]
        in_maps.append({"lbst": lbst, "wtg": wtg, "cst": cst})
    return in_maps


LAST_RESULT = None


def kernel(**inputs) -> np.ndarray:
    global LAST_RESULT
    inputs = {k: np.asarray(v) for k, v in inputs.items()}
    nc = _build()
    in_maps = make_in_maps(inputs)
    res = bass_utils.run_bass_kernel_spmd(
        nc,
        in_maps,
        core_ids=list(range(B)),
        trace=os.environ.get("LFE_TRACE", "0") == "1",
    )
    LAST_RESULT = res
    out = np.ascontiguousarray(
        np.stack([res.results[b]["out"] for b in range(B)], axis=0)
        .transpose(0, 2, 1)
    )
    return out


if __name__ == "__main__":
    rng = np.random.default_rng(0)
    inputs = {
        "shape_code": rng.standard_normal((B, 64), dtype=np.float32),
        "structure_code": rng.standard_normal((B, 32), dtype=np.float32),
        "pose_code": rng.standard_normal((B, 32), dtype=np.float32),
        "lbs_weights": rng.random((B, T, K), dtype=np.float32),
        "W": rng.standard_normal((K, F, Z), dtype=np.float32),
        "bias": rng.standard_normal((K, F), dtype=np.float32),
    }
    out = kernel(**inputs)
    g = np.concatenate(
        [inputs["shape_code"], inputs["structure_code"], inputs["pose_code"]], -1
    )
    local = np.einsum("kfz,bz->bkf", inputs["W"], g) + inputs["bias"][None]
    ref = np.einsum("btk,bkf->btf", inputs["lbs_weights"], local)
    err = np.abs(out - ref).max() / np.abs(ref).max()
    print("rel err:", err)


# revision 7
# speedup vs baseline: 1.0295x; 1.0266x over previous
"""Trainium2 Bass kernel for nn_LocalFeatureEncoder — v2 (transposed dataflow).

Computes, for B=8 batches on 8 NeuronCores (batch b -> core b):
    g      = concat(shape_code, structure_code, pose_code)      # (B, 128)
    local  = einsum('kfz,bz->bkf', W, g) + bias                 # (B, 24, 64)
    out    = einsum('btk,bkf->btf', lbs_weights, local)         # (B, 32768, 64)

Host pre-transposes lbs to lbsT2 [48, 16384] bf16 (two T-halves of lbs^T
stacked on the partition axis); output is produced as out^T (64, 32768) f32
(f on partitions) and transposed back on host. Device program:

  Stage 1 (overlapped with the lbs stream, no DMA after the const loads;
  W is loaded as two half-DMAs so the matvec chain starts after the first):
  24 PE matvecs with per-joint W tiles laid out so PSUM collects
  localcol [64, 24] = local^T in (f-partition, k-col) layout, DVE bias-add,
  one PE transpose -> localT [24, 64], cast-copy to bf16 SBUF.

  Stage 2: per 512-col chunk, TWO matmuls (one per T-half, operand
  partition bases 0/32) fill psum[128, 512] (partitions 0..63 = f half 0,
  64..127 = f half 1), PSUM->SBUF copies alternating DVE/Act, 18 output
  DMAs split over the SP/Act queues with tuned staging-group sizes. One
  PE warmup matmul keeps the pstate ramp warm while the localT copy
  completes.
"""

import os
from contextlib import ExitStack

import numpy as np
import ml_dtypes

import concourse.bass as bass
import concourse.bacc as bacc
import concourse.tile as tile
from concourse import mybir
from concourse import bass_utils

BF16 = ml_dtypes.bfloat16

B, T, K, Z, F = 8, 32768, 24, 128, 64
TH = T // 2             # 16384 t-cols per half
KK = 2 * K              # 48 partitions of lbsT
CW = 512                # t-cols per matmul (1 PSUM bank)
NC = TH // CW           # 32 chunks
C_BIAS, C_ID = 0, 24    # f32 const layout: [biasT 24 | ident 64]
C_TOT = 88
_GROUPS = (3, 3, 4, 4, 4, 4, 4, 3, 3)  # staging group sizes, sum = NC
_BOUNDS = (0, 1024, 6144, 11264, TH)  # lbs input chunk boundaries (cols)
_SPLIT_IN = False
_HEADQ = 0  # head DMA queue assignment variant  # True: 2 DMAs/chunk skipping pad rows (less bytes, more issues)

_built = {}


def _build(key=0):
    if key in _built:
        return _built[key]

    f32 = mybir.dt.float32
    bf16 = mybir.dt.bfloat16
    nc = bacc.Bacc("TRN2", target_bir_lowering=False, debug=False)

    lbst_d = nc.dram_tensor("lbst", (56, TH), bf16, kind="ExternalInput")
    wtg_d = nc.dram_tensor("wtg", (128, K * F + 1), bf16, kind="ExternalInput")
    cst_d = nc.dram_tensor("cst", (64, C_TOT), f32, kind="ExternalInput")
    out_d = nc.dram_tensor("out", (F, T), f32, kind="ExternalOutput")

    with tile.TileContext(nc) as tc, ExitStack() as ctx:
        const = ctx.enter_context(tc.tile_pool(name="const", bufs=1))
        big = ctx.enter_context(tc.tile_pool(name="big", bufs=1))
        ps1 = ctx.enter_context(
            tc.tile_pool(name="ps1", bufs=1, space=bass.MemorySpace.PSUM)
        )
        psO = ctx.enter_context(
            tc.tile_pool(name="psO", bufs=6, space=bass.MemorySpace.PSUM)
        )
        stag_pool = ctx.enter_context(tc.tile_pool(name="stag_pool", bufs=6))

        # ---- input DMAs (SP queue): consts first, then lbs chunks ----
        # wtg layout: [wt tiles 0..11 | g | wt tiles 12..23]; loaded as two
        # DMAs so the stage-1 matvec chain starts after the first half
        HW1 = 12 * F + 1
        wtg = const.tile([128, K * F + 1], bf16)
        cst = const.tile([64, C_TOT], f32)
        if _HEADQ == 0:      # wtg1+wtg2 on SP, cst on Act
            nc.sync.dma_start(wtg[:, 0:HW1], wtg_d.ap()[:, 0:HW1])
            nc.sync.dma_start(wtg[:, HW1:], wtg_d.ap()[:, HW1:])
            nc.scalar.dma_start(cst[:], cst_d.ap())
        elif _HEADQ == 1:    # wtg2 then cst on Act
            nc.sync.dma_start(wtg[:, 0:HW1], wtg_d.ap()[:, 0:HW1])
            nc.scalar.dma_start(wtg[:, HW1:], wtg_d.ap()[:, HW1:])
            nc.scalar.dma_start(cst[:], cst_d.ap())
        else:                # cst then wtg2 on Act
            nc.sync.dma_start(wtg[:, 0:HW1], wtg_d.ap()[:, 0:HW1])
            nc.scalar.dma_start(cst[:], cst_d.ap())
            nc.scalar.dma_start(wtg[:, HW1:], wtg_d.ap()[:, HW1:])

        lbst_sb = big.tile([56, TH], bf16)
        bounds = list(_BOUNDS)
        for c in range(len(bounds) - 1):
            lo, hi = bounds[c], bounds[c + 1]
            nc.sync.dma_start(lbst_sb[:, lo:hi], lbst_d.ap()[:, lo:hi])

        g_col = wtg[:, 12 * F:12 * F + 1]
        biasT = cst[:, C_BIAS:C_BIAS + K]  # [64, 24] = bias^T
        ident = cst[:, C_ID:C_ID + 64]     # [64, 64]

        # ---- stage 1: localT = (W @ g + bias)^T -> [24, 64] bf16, no DMA ----
        lc_ps = ps1.tile([64, K], f32, tag="s1")
        for j in range(K):
            off = j * F if j < 12 else HW1 + (j - 12) * F
            nc.tensor.matmul(
                lc_ps[:, j:j + 1], wtg[:, off:off + F], g_col,
                start=True, stop=True,
            )
        # bias-add writes the local^T columns twice (cols 0..24 and 32..56) so
        # a single base-0 transpose yields localT duplicated at partition
        # bases 0 and 32 — each T-half matmul then has base-aligned lhsT/rhs
        localcol = const.tile([64, 32 + K], f32)
        nc.vector.memset(localcol[:, K:32], 0.0)
        nc.vector.tensor_add(localcol[:, 0:K], lc_ps[:], biasT)
        nc.vector.tensor_add(localcol[:, 32:32 + K], lc_ps[:], biasT)

        lT_ps = ps1.tile([32 + K, F], f32, tag="s1")
        nc.tensor.transpose(lT_ps[:], localcol[:], ident)

        # keep the PE busy while the localT copy completes so the pstate ramp
        # (low->mid->max after 3us continuous) is warm when stage 2 starts
        warm_ps = ps1.tile([128, CW], f32, tag="warm")
        nc.tensor.matmul(
            warm_ps[:], wtg[:, 0:128], wtg[:, 0:CW], start=True, stop=True,
        )

        localT = const.tile([32 + K, F], bf16)
        nc.vector.tensor_copy(localT[:], lT_ps[:])

        # ---- stage 2: 64 matmuls + 32 copies + 16 output DMAs ----
        # ramped group sizes: small groups first to minimize pipeline-fill
        # latency, large groups in steady state to minimize DMA issue count
        group_sizes = list(_GROUPS)
        assert sum(group_sizes) == NC
        c = 0
        for gi, gs in enumerate(group_sizes):
            stag = stag_pool.tile([128, gs * CW], f32)
            lo = c
            for i in range(gs):
                ops = psO.tile([128, CW], f32)
                nc.tensor.matmul(
                    ops[0:F, :], localT[0:K, :],
                    lbst_sb[0:K, c * CW:(c + 1) * CW],
                    start=True, stop=True,
                )
                nc.tensor.matmul(
                    ops[F:2 * F, :], localT[32:32 + K, :],
                    lbst_sb[32:32 + K, c * CW:(c + 1) * CW],
                    start=True, stop=True,
                )
                if c % 2 == 0:
                    nc.vector.tensor_copy(stag[:, i * CW:(i + 1) * CW], ops[:])
                else:
                    nc.scalar.copy(stag[:, i * CW:(i + 1) * CW], ops[:])
                c += 1
            h0 = out_d.ap()[:, lo * CW:c * CW]
            h1 = out_d.ap()[:, TH + lo * CW:TH + c * CW]
            if gi % 2 == 0:
                nc.sync.dma_start(h0, stag[0:F, :])
                nc.scalar.dma_start(h1, stag[F:2 * F, :])
            else:
                nc.scalar.dma_start(h0, stag[0:F, :])
                nc.sync.dma_start(h1, stag[F:2 * F, :])

    nc.compile()
    _built[key] = nc
    return nc


def make_in_maps(inputs):
    g_full = np.concatenate(
        [inputs["shape_code"], inputs["structure_code"], inputs["pose_code"]],
        axis=-1,
    ).astype(np.float32)  # (8, 128)
    W = inputs["W"].astype(np.float32)
    # wt[z, k*F + f] = W[k, f, z]
    wt = np.ascontiguousarray(W.transpose(2, 0, 1).reshape(Z, K * F))
    cst = np.zeros((64, C_TOT), dtype=np.float32)
    cst[:, C_BIAS:C_BIAS + K] = inputs["bias"].astype(np.float32).T
    cst[:, C_ID:C_ID + 64] = np.eye(64, dtype=np.float32)
    lbs_bf = inputs["lbs_weights"].astype(BF16)
    in_maps = []
    for b in range(B):
        wtg = np.empty((128, K * F + 1), dtype=BF16)
        wtg[:, 0:12 * F] = wt[:, 0:12 * F].astype(BF16)
        wtg[:, 12 * F] = g_full[b].astype(BF16)
        wtg[:, 12 * F + 1:] = wt[:, 12 * F:].astype(BF16)
        lbst2 = lbs_bf[b].reshape(2, TH, K).transpose(0, 2, 1)  # (2, K, TH)
        lbst = np.zeros((56, TH), dtype=BF16)
        lbst[0:K] = lbst2[0]
        lbst[32:32 + K] = lbst2[1]
        in_maps.append({"lbst": lbst, "wtg": wtg, "cst": cst})
    return in_maps


LAST_RESULT = None


def kernel(**inputs) -> np.ndarray:
    global LAST_RESULT
    inputs = {k: np.asarray(v) for k, v in inputs.items()}
    nc = _build()
    in_maps = make_in_maps(inputs)
    res = bass_utils.run_bass_kernel_spmd(
        nc,
        in_maps,
        core_ids=list(range(B)),
        trace=os.environ.get("LFE_TRACE", "0") == "1",
    )
    LAST_RESULT = res
    out = np.ascontiguousarray(
        np.stack([res.results[b]["out"] for b in range(B)], axis=0)
        .transpose(0, 2, 1)
    )
    return out


if __name__ == "__main__":
    rng = np.random.default_rng(0)
    inputs = {
        "shape_code": rng.standard_normal((B, 64), dtype=np.float32),
        "structure_code": rng.standard_normal((B, 32), dtype=np.float32),
        "pose_code": rng.standard_normal((B, 32), dtype=np.float32),
        "lbs_weights": rng.random((B, T, K), dtype=np.float32),
        "W": rng.standard_normal((K, F, Z), dtype=np.float32),
        "bias": rng.standard_normal((K, F), dtype=np.float32),
    }
    out = kernel(**inputs)
    g = np.concatenate(
        [inputs["shape_code"], inputs["structure_code"], inputs["pose_code"]], -1
    )
    local = np.einsum("kfz,bz->bkf", inputs["W"], g) + inputs["bias"][None]
    ref = np.einsum("btk,bkf->btf", inputs["lbs_weights"], local)
    err = np.abs(out - ref).max() / np.abs(ref).max()
    print("rel err:", err)


# revision 8
# speedup vs baseline: 1.0465x; 1.0165x over previous
"""Trainium2 Bass kernel for nn_LocalFeatureEncoder — v2 (transposed dataflow).

Computes, for B=8 batches on 8 NeuronCores (batch b -> core b):
    g      = concat(shape_code, structure_code, pose_code)      # (B, 128)
    local  = einsum('kfz,bz->bkf', W, g) + bias                 # (B, 24, 64)
    out    = einsum('btk,bkf->btf', lbs_weights, local)         # (B, 32768, 64)

Host pre-transposes lbs to lbsT2 [48, 16384] bf16 (two T-halves of lbs^T
stacked on the partition axis); output is produced as out^T (64, 32768) f32
(f on partitions) and transposed back on host. Device program:

  Stage 1 (overlapped with the lbs stream; W loaded as two half-DMAs so
  the matvec chain starts after the first): 128 PE matvecs whose lhsT
  tiles are host-packed per f-column so PSUM collects localT DIRECTLY in
  its final [56, 64] dup layout (rows 0..24 and 32..56 = local^T at
  partition bases 0/32); two DVE bias-adds are the PSUM->SBUF
  materialization — no transpose, no identity, no extra copy.

  Stage 2: per 512-col chunk, TWO matmuls (one per T-half, operand
  partition bases 0/32) fill psum[128, 512] (partitions 0..63 = f half 0,
  64..127 = f half 1), PSUM->SBUF copies alternating DVE/Act, 18 output
  DMAs split over the SP/Act queues with tuned staging-group sizes.
"""

import os
from contextlib import ExitStack

import numpy as np
import ml_dtypes

import concourse.bass as bass
import concourse.bacc as bacc
import concourse.tile as tile
from concourse import mybir
from concourse import bass_utils

BF16 = ml_dtypes.bfloat16

B, T, K, Z, F = 8, 32768, 24, 128, 64
TH = T // 2             # 16384 t-cols per half
KK = 2 * K              # 48 partitions of lbsT
CW = 512                # t-cols per matmul (1 PSUM bank)
NC = TH // CW           # 32 chunks
C_BIAS, C_ID = 0, 24    # f32 const layout: [biasT 24 | ident 64]
C_TOT = 88
_GROUPS = (3, 3, 4, 4, 4, 4, 4, 3, 3)  # staging group sizes, sum = NC
_BOUNDS = (0, 1024, 6144, 11264, TH)  # lbs input chunk boundaries (cols)
_SPLIT_IN = False
_HEADQ = 0  # head DMA queue assignment variant
_WS = 12   # W tiles in the first wtg DMA  # True: 2 DMAs/chunk skipping pad rows (less bytes, more issues)

_built = {}


def _build(key=0):
    if key in _built:
        return _built[key]

    f32 = mybir.dt.float32
    bf16 = mybir.dt.bfloat16
    nc = bacc.Bacc("TRN2", target_bir_lowering=False, debug=False)

    lbst_d = nc.dram_tensor("lbst", (56, TH), bf16, kind="ExternalInput")
    wtg_d = nc.dram_tensor("wtg", (128, 64 * K + 1 + F), bf16, kind="ExternalInput")
    out_d = nc.dram_tensor("out", (F, T), f32, kind="ExternalOutput")

    with tile.TileContext(nc) as tc, ExitStack() as ctx:
        const = ctx.enter_context(tc.tile_pool(name="const", bufs=1))
        big = ctx.enter_context(tc.tile_pool(name="big", bufs=1))
        ps1 = ctx.enter_context(
            tc.tile_pool(name="ps1", bufs=1, space=bass.MemorySpace.PSUM)
        )
        psO = ctx.enter_context(
            tc.tile_pool(name="psO", bufs=6, space=bass.MemorySpace.PSUM)
        )
        stag_pool = ctx.enter_context(tc.tile_pool(name="stag_pool", bufs=6))

        # ---- input DMAs (SP queue): consts first, then lbs chunks ----
        # wtg layout: [wt_f tiles 0..31 (24 cols each) | g | tiles 32..63 |
        # biasT2 64]; two DMAs so matvecs for f<32 start after the first.
        # wt_f tile c holds lhsT [z, 24] with col k = W[k, c, z]; biasT2
        # [56-row, 64] holds bias in localT's dup layout.
        HW1 = 32 * K + 1
        WTOT = 64 * K + 1 + F
        wtg = const.tile([128, WTOT], bf16)
        nc.sync.dma_start(wtg[:, 0:HW1], wtg_d.ap()[:, 0:HW1])
        nc.sync.dma_start(wtg[:, HW1:], wtg_d.ap()[:, HW1:])

        lbst_sb = big.tile([56, TH], bf16)
        bounds = list(_BOUNDS)
        for c in range(len(bounds) - 1):
            lo, hi = bounds[c], bounds[c + 1]
            nc.sync.dma_start(lbst_sb[:, lo:hi], lbst_d.ap()[:, lo:hi])

        g_col = wtg[:, 32 * K:32 * K + 1]
        bias2 = wtg[:, 64 * K + 1:64 * K + 1 + F]  # rows 0..56 = dup layout

        # ---- stage 1: localT built DIRECTLY in its final [56, 64] dup
        # layout: two matvecs per f-column (psum partition bases 0 and 32),
        # lhsT = wt_f tile c [z, 24] with col k = W[k, c, z]. The bias-add
        # is then the PSUM->SBUF materialization — no transpose, no ident,
        # no extra copy.
        lT_ps = ps1.tile([32 + K, F], f32, tag="s1")
        for c0 in range(F):
            off = c0 * K if c0 < 32 else HW1 + (c0 - 32) * K
            nc.tensor.matmul(
                lT_ps[0:K, c0:c0 + 1], wtg[:, off:off + K], g_col,
                start=True, stop=True,
            )
            nc.tensor.matmul(
                lT_ps[32:32 + K, c0:c0 + 1], wtg[:, off:off + K], g_col,
                start=True, stop=True,
            )

        localT = const.tile([32 + K, F], bf16)
        nc.vector.tensor_add(localT[0:K, :], lT_ps[0:K, :], bias2[0:K, :])
        nc.vector.tensor_add(
            localT[32:32 + K, :], lT_ps[32:32 + K, :], bias2[32:32 + K, :]
        )

        # ---- stage 2: 64 matmuls + 32 copies + 16 output DMAs ----
        # ramped group sizes: small groups first to minimize pipeline-fill
        # latency, large groups in steady state to minimize DMA issue count
        group_sizes = list(_GROUPS)
        assert sum(group_sizes) == NC
        c = 0
        for gi, gs in enumerate(group_sizes):
            stag = stag_pool.tile([128, gs * CW], f32)
            lo = c
            for i in range(gs):
                ops = psO.tile([128, CW], f32)
                nc.tensor.matmul(
                    ops[0:F, :], localT[0:K, :],
                    lbst_sb[0:K, c * CW:(c + 1) * CW],
                    start=True, stop=True,
                )
                nc.tensor.matmul(
                    ops[F:2 * F, :], localT[32:32 + K, :],
                    lbst_sb[32:32 + K, c * CW:(c + 1) * CW],
                    start=True, stop=True,
                )
                if c % 2 == 0:
                    nc.vector.tensor_copy(stag[:, i * CW:(i + 1) * CW], ops[:])
                else:
                    nc.scalar.copy(stag[:, i * CW:(i + 1) * CW], ops[:])
                c += 1
            h0 = out_d.ap()[:, lo * CW:c * CW]
            h1 = out_d.ap()[:, TH + lo * CW:TH + c * CW]
            if gi % 2 == 0:
                nc.sync.dma_start(h0, stag[0:F, :])
                nc.scalar.dma_start(h1, stag[F:2 * F, :])
            else:
                nc.scalar.dma_start(h0, stag[0:F, :])
                nc.sync.dma_start(h1, stag[F:2 * F, :])

    nc.compile()
    _built[key] = nc
    return nc


def make_in_maps(inputs):
    g_full = np.concatenate(
        [inputs["shape_code"], inputs["structure_code"], inputs["pose_code"]],
        axis=-1,
    ).astype(np.float32)  # (8, 128)
    W = inputs["W"].astype(np.float32)
    # wt_f[z, c*K + k] = W[k, c, z]
    wt = np.ascontiguousarray(W.transpose(2, 1, 0).reshape(Z, F * K))
    bias2 = np.zeros((128, F), dtype=np.float32)
    bias2[0:K] = inputs["bias"].astype(np.float32)
    bias2[32:32 + K] = inputs["bias"].astype(np.float32)
    lbs_bf = inputs["lbs_weights"].astype(BF16)
    in_maps = []
    for b in range(B):
        wtg = np.empty((128, F * K + 1 + F), dtype=BF16)
        wtg[:, 0:32 * K] = wt[:, 0:32 * K].astype(BF16)
        wtg[:, 32 * K] = g_full[b].astype(BF16)
        wtg[:, 32 * K + 1:64 * K + 1] = wt[:, 32 * K:].astype(BF16)
        wtg[:, 64 * K + 1:] = bias2.astype(BF16)
        lbst2 = lbs_bf[b].reshape(2, TH, K).transpose(0, 2, 1)  # (2, K, TH)
        lbst = np.zeros((56, TH), dtype=BF16)
        lbst[0:K] = lbst2[0]
        lbst[32:32 + K] = lbst2[1]
        in_maps.append({"lbst": lbst, "wtg": wtg})
    return in_maps


LAST_RESULT = None


def kernel(**inputs) -> np.ndarray:
    global LAST_RESULT
    inputs = {k: np.asarray(v) for k, v in inputs.items()}
    nc = _build()
    in_maps = make_in_maps(inputs)
    res = bass_utils.run_bass_kernel_spmd(
        nc,
        in_maps,
        core_ids=list(range(B)),
        trace=os.environ.get("LFE_TRACE", "0") == "1",
    )
    LAST_RESULT = res
    out = np.ascontiguousarray(
        np.stack([res.results[b]["out"] for b in range(B)], axis=0)
        .transpose(0, 2, 1)
    )
    return out


if __name__ == "__main__":
    rng = np.random.default_rng(0)
    inputs = {
        "shape_code": rng.standard_normal((B, 64), dtype=np.float32),
        "structure_code": rng.standard_normal((B, 32), dtype=np.float32),
        "pose_code": rng.standard_normal((B, 32), dtype=np.float32),
        "lbs_weights": rng.random((B, T, K), dtype=np.float32),
        "W": rng.standard_normal((K, F, Z), dtype=np.float32),
        "bias": rng.standard_normal((K, F), dtype=np.float32),
    }
    out = kernel(**inputs)
    g = np.concatenate(
        [inputs["shape_code"], inputs["structure_code"], inputs["pose_code"]], -1
    )
    local = np.einsum("kfz,bz->bkf", inputs["W"], g) + inputs["bias"][None]
    ref = np.einsum("btk,bkf->btf", inputs["lbs_weights"], local)
    err = np.abs(out - ref).max() / np.abs(ref).max()
    print("rel err:", err)


# revision 9
# speedup vs baseline: 1.0572x; 1.0102x over previous
"""Trainium2 Bass kernel for nn_LocalFeatureEncoder — v2 (transposed dataflow).

Computes, for B=8 batches on 8 NeuronCores (batch b -> core b):
    g      = concat(shape_code, structure_code, pose_code)      # (B, 128)
    local  = einsum('kfz,bz->bkf', W, g) + bias                 # (B, 24, 64)
    out    = einsum('btk,bkf->btf', lbs_weights, local)         # (B, 32768, 64)

Host pre-transposes lbs to lbsT2 [48, 16384] bf16 (two T-halves of lbs^T
stacked on the partition axis); output is produced as out^T (64, 32768) f32
(f on partitions) and transposed back on host. Device program:

  Stage 1 (overlapped with the lbs stream; W loaded as two half-DMAs so
  the matvec chain starts after the first): 128 PE matvecs whose lhsT
  tiles are host-packed per f-column so PSUM collects localT DIRECTLY in
  its final [56, 64] dup layout (rows 0..24 and 32..56 = local^T at
  partition bases 0/32); two DVE bias-adds are the PSUM->SBUF
  materialization — no transpose, no identity, no extra copy.

  Stage 2: per 512-col chunk, TWO matmuls (one per T-half, operand
  partition bases 0/32) fill psum[128, 512] (partitions 0..63 = f half 0,
  64..127 = f half 1), PSUM->SBUF copies alternating DVE/Act, 18 output
  DMAs split over the SP/Act queues with tuned staging-group sizes.
"""

import os
from contextlib import ExitStack

import numpy as np
import ml_dtypes

import concourse.bass as bass
import concourse.bacc as bacc
import concourse.tile as tile
from concourse import mybir
from concourse import bass_utils

BF16 = ml_dtypes.bfloat16

B, T, K, Z, F = 8, 32768, 24, 128, 64
TH = T // 2             # 16384 t-cols per half
KK = 2 * K              # 48 partitions of lbsT
CW = 512                # t-cols per matmul (1 PSUM bank)
NC = TH // CW           # 32 chunks
C_BIAS, C_ID = 0, 24    # f32 const layout: [biasT 24 | ident 64]
C_TOT = 88
_GROUPS = (2, 3, 4, 4, 4, 4, 4, 4, 3)  # staging group sizes, sum = NC
_BOUNDS = (0, 1024, 6144, 11264, TH)  # lbs input chunk boundaries (cols)
_SPLIT_IN = False
_HEADQ = 0  # head DMA queue assignment variant
_WS = 12   # W tiles in the first wtg DMA  # True: 2 DMAs/chunk skipping pad rows (less bytes, more issues)

_built = {}


def _build(key=0):
    if key in _built:
        return _built[key]

    f32 = mybir.dt.float32
    bf16 = mybir.dt.bfloat16
    nc = bacc.Bacc("TRN2", target_bir_lowering=False, debug=False)

    lbst_d = nc.dram_tensor("lbst", (56, TH), bf16, kind="ExternalInput")
    wtg_d = nc.dram_tensor("wtg", (128, 64 * K + 1 + F), bf16, kind="ExternalInput")
    out_d = nc.dram_tensor("out", (F, T), f32, kind="ExternalOutput")

    with tile.TileContext(nc) as tc, ExitStack() as ctx:
        const = ctx.enter_context(tc.tile_pool(name="const", bufs=1))
        big = ctx.enter_context(tc.tile_pool(name="big", bufs=1))
        ps1 = ctx.enter_context(
            tc.tile_pool(name="ps1", bufs=1, space=bass.MemorySpace.PSUM)
        )
        psO = ctx.enter_context(
            tc.tile_pool(name="psO", bufs=6, space=bass.MemorySpace.PSUM)
        )
        stag_pool = ctx.enter_context(tc.tile_pool(name="stag_pool", bufs=6))

        # ---- input DMAs (SP queue): consts first, then lbs chunks ----
        # wtg layout: [wt_f tiles 0..31 (24 cols each) | g | tiles 32..63 |
        # biasT2 64]; two DMAs so matvecs for f<32 start after the first.
        # wt_f tile c holds lhsT [z, 24] with col k = W[k, c, z]; biasT2
        # [56-row, 64] holds bias in localT's dup layout.
        HW1 = 32 * K + 1
        WTOT = 64 * K + 1 + F
        wtg = const.tile([128, WTOT], bf16)
        nc.sync.dma_start(wtg[:, 0:HW1], wtg_d.ap()[:, 0:HW1])
        nc.sync.dma_start(wtg[:, HW1:], wtg_d.ap()[:, HW1:])

        lbst_sb = big.tile([56, TH], bf16)
        bounds = list(_BOUNDS)
        for c in range(len(bounds) - 1):
            lo, hi = bounds[c], bounds[c + 1]
            nc.sync.dma_start(lbst_sb[:, lo:hi], lbst_d.ap()[:, lo:hi])

        g_col = wtg[:, 32 * K:32 * K + 1]
        bias2 = wtg[:, 64 * K + 1:64 * K + 1 + F]  # rows 0..56 = dup layout

        # ---- stage 1: localT built DIRECTLY in its final [56, 64] dup
        # layout: two matvecs per f-column (psum partition bases 0 and 32),
        # lhsT = wt_f tile c [z, 24] with col k = W[k, c, z]. The bias-add
        # is then the PSUM->SBUF materialization — no transpose, no ident,
        # no extra copy.
        lT_ps = ps1.tile([32 + K, F], f32, tag="s1")
        for c0 in range(F):
            off = c0 * K if c0 < 32 else HW1 + (c0 - 32) * K
            nc.tensor.matmul(
                lT_ps[0:K, c0:c0 + 1], wtg[:, off:off + K], g_col,
                start=True, stop=True,
            )
            nc.tensor.matmul(
                lT_ps[32:32 + K, c0:c0 + 1], wtg[:, off:off + K], g_col,
                start=True, stop=True,
            )

        localT = const.tile([32 + K, F], bf16)
        nc.vector.tensor_add(localT[0:K, :], lT_ps[0:K, :], bias2[0:K, :])
        nc.vector.tensor_add(
            localT[32:32 + K, :], lT_ps[32:32 + K, :], bias2[32:32 + K, :]
        )

        # ---- stage 2: 64 matmuls + 32 copies + 16 output DMAs ----
        # ramped group sizes: small groups first to minimize pipeline-fill
        # latency, large groups in steady state to minimize DMA issue count
        group_sizes = list(_GROUPS)
        assert sum(group_sizes) == NC
        c = 0
        for gi, gs in enumerate(group_sizes):
            stag = stag_pool.tile([128, gs * CW], f32)
            lo = c
            for i in range(gs):
                ops = psO.tile([128, CW], f32)
                nc.tensor.matmul(
                    ops[0:F, :], localT[0:K, :],
                    lbst_sb[0:K, c * CW:(c + 1) * CW],
                    start=True, stop=True,
                )
                nc.tensor.matmul(
                    ops[F:2 * F, :], localT[32:32 + K, :],
                    lbst_sb[32:32 + K, c * CW:(c + 1) * CW],
                    start=True, stop=True,
                )
                if c % 2 == 0:
                    nc.vector.tensor_copy(stag[:, i * CW:(i + 1) * CW], ops[:])
                else:
                    nc.scalar.copy(stag[:, i * CW:(i + 1) * CW], ops[:])
                c += 1
            h0 = out_d.ap()[:, lo * CW:c * CW]
            h1 = out_d.ap()[:, TH + lo * CW:TH + c * CW]
            if gi % 2 == 0:
                nc.sync.dma_start(h0, stag[0:F, :])
                nc.scalar.dma_start(h1, stag[F:2 * F, :])
            else:
                nc.scalar.dma_start(h0, stag[0:F, :])
                nc.sync.dma_start(h1, stag[F:2 * F, :])

    nc.compile()
    _built[key] = nc
    return nc


def make_in_maps(inputs):
    g_full = np.concatenate(
        [inputs["shape_code"], inputs["structure_code"], inputs["pose_code"]],
        axis=-1,
    ).astype(np.float32)  # (8, 128)
    W = inputs["W"].astype(np.float32)
    # wt_f[z, c*K + k] = W[k, c, z]
    wt = np.ascontiguousarray(W.transpose(2, 1, 0).reshape(Z, F * K))
    bias2 = np.zeros((128, F), dtype=np.float32)
    bias2[0:K] = inputs["bias"].astype(np.float32)
    bias2[32:32 + K] = inputs["bias"].astype(np.float32)
    lbs_bf = inputs["lbs_weights"].astype(BF16)
    in_maps = []
    for b in range(B):
        wtg = np.empty((128, F * K + 1 + F), dtype=BF16)
        wtg[:, 0:32 * K] = wt[:, 0:32 * K].astype(BF16)
        wtg[:, 32 * K] = g_full[b].astype(BF16)
        wtg[:, 32 * K + 1:64 * K + 1] = wt[:, 32 * K:].astype(BF16)
        wtg[:, 64 * K + 1:] = bias2.astype(BF16)
        lbst2 = lbs_bf[b].reshape(2, TH, K).transpose(0, 2, 1)  # (2, K, TH)
        lbst = np.zeros((56, TH), dtype=BF16)
        lbst[0:K] = lbst2[0]
        lbst[32:32 + K] = lbst2[1]
        in_maps.append({"lbst": lbst, "wtg": wtg})
    return in_maps


LAST_RESULT = None


def kernel(**inputs) -> np.ndarray:
    global LAST_RESULT
    inputs = {k: np.asarray(v) for k, v in inputs.items()}
    nc = _build()
    in_maps = make_in_maps(inputs)
    res = bass_utils.run_bass_kernel_spmd(
        nc,
        in_maps,
        core_ids=list(range(B)),
        trace=os.environ.get("LFE_TRACE", "0") == "1",
    )
    LAST_RESULT = res
    out = np.ascontiguousarray(
        np.stack([res.results[b]["out"] for b in range(B)], axis=0)
        .transpose(0, 2, 1)
    )
    return out


if __name__ == "__main__":
    rng = np.random.default_rng(0)
    inputs = {
        "shape_code": rng.standard_normal((B, 64), dtype=np.float32),
        "structure_code": rng.standard_normal((B, 32), dtype=np.float32),
        "pose_code": rng.standard_normal((B, 32), dtype=np.float32),
        "lbs_weights": rng.random((B, T, K), dtype=np.float32),
        "W": rng.standard_normal((K, F, Z), dtype=np.float32),
        "bias": rng.standard_normal((K, F), dtype=np.float32),
    }
    out = kernel(**inputs)
    g = np.concatenate(
        [inputs["shape_code"], inputs["structure_code"], inputs["pose_code"]], -1
    )
    local = np.einsum("kfz,bz->bkf", inputs["W"], g) + inputs["bias"][None]
    ref = np.einsum("btk,bkf->btf", inputs["lbs_weights"], local)
    err = np.abs(out - ref).max() / np.abs(ref).max()
    print("rel err:", err)
